# revision 11
# baseline (speedup 1.0000x reference)
"""Trainium2 Bass kernel for nn_Channel: per-row complex FIR (overlap-save
DFT-64 on the TensorEngine) + 64-point DFT of the taps (H_t).

Contract: kernel(**inputs) takes the FULL unsharded inputs
  input: (256, 64, 2048, 2) f32, cof: (256, 64, 32, 2) f32, M: 64
returns (output, H_t) with
  output: (256, 64, 2079, 2) f32, H_t: (256, 64, 64, 2) f32
Internally shards the batch dim N=256 across 8 NeuronCores (data parallel).

Algorithm (per row = one (n, p) pair; 2048 rows per core):
  y = conv_full(x, h) via overlap-save with FFT size 64 and hop 32:
    block b covers output times [32b, 32b+32); its input window is
    x[32b-32 .. 32b+32) (64 samples).  DFT-64 / IDFT-64 are matmuls with
    constant real-packed [128x128] weight matrices on the PE; the per-row
    spectrum product S*H is 2 elementwise tensor_tensor ops on the DVE
    (t_a = S*[Hr;Hi], t_b = S*[Hi;Hr]); the +/- recombination is folded
    into the constant IDFT weights (Wa, Wb).
  H_t = DFT-64 of zero-padded taps, computed in fp32 exactly.

Layouts: time-on-partitions via one DMA-xbar transpose per row-tile
(2-byte dtype => fp16 compute path), and one transpose back for the
output.  Odd blocks straddle 128-element chunks and are handled as two
half-K (K=64) matmuls accumulating in PSUM.
"""

import os
import sys
from contextlib import ExitStack

import numpy as np

for _p in (
    "/root/.axon_site",
    "/root/.axon_site/_ro/trn_rl_repo",
    "/root/.axon_site/_ro/pypackages",
    "/opt/trn_rl_repo",
):
    if os.path.isdir(_p) and _p not in sys.path:
        sys.path.append(_p)

import concourse.bass as bass  # noqa: E402
import concourse.tile as tile  # noqa: E402
from concourse import bacc, mybir  # noqa: E402

F32 = mybir.dt.float32
F16 = mybir.dt.float16

N_CORES = 8
N_FULL, P_DIM, SMK, L, M_FFT = 256, 64, 2048, 32, 64
N_PER = N_FULL // N_CORES  # 32
ROWS_PER_CORE = N_PER * P_DIM  # 2048
OUT_T = SMK + L - 1  # 2079
NB = 65  # overlap-save blocks per row (hop 32)
NCH = 33  # 128-element chunks in the padded row
RME = NCH * 128  # 4224 padded row elements (= 2*(2048+64) )
SG = 8  # blocks per supergroup (2 PSUM banks of S, 1 of O)


def _constants():
    """Constant weight matrices (numpy, fed as ExternalInputs)."""
    f = np.arange(M_FFT)
    t = np.arange(M_FFT)
    th = 2.0 * np.pi * np.outer(t, f) / M_FFT  # [t, f]
    c, s = np.cos(th), np.sin(th)
    # Forward DFT, real-packed.  Row 2t+comp, col 2f+comp'.
    # S_r = sum xr cos + xi sin ; S_i = sum xi cos - xr sin
    wf = np.zeros((128, 128), np.float32)
    wf[0::2, 0::2] = c
    wf[1::2, 0::2] = s
    wf[0::2, 1::2] = -s
    wf[1::2, 1::2] = c
    # Taps DFT (H), same structure but rows are (l, comp), l < 32 (rest zero).
    wh_a = np.zeros((128, 128), np.float32)
    wh_a[0::2, 0::2] = c
    wh_a[1::2, 0::2] = s
    wh_a[0::2, 1::2] = -s
    wh_a[1::2, 1::2] = c
    wh_a[64:, :] = 0.0
    # Swapped variant: col 2f+0 = H_i, col 2f+1 = H_r
    wh_b = np.zeros((128, 128), np.float32)
    wh_b[0::2, 0::2] = -s
    wh_b[1::2, 0::2] = c
    wh_b[0::2, 1::2] = c
    wh_b[1::2, 1::2] = s
    wh_b[64:, :] = 0.0

    # IDFT keep-half weights.  t_a[2f+c'] = (SrHr, SiHi), t_b = (SrHi, SiHr).
    # S'_r = ta[2f]-ta[2f+1], S'_i = tb[2f]+tb[2f+1]
    # y_r[j] = 1/64 sum_f S'_r cos - S'_i sin ; y_i = 1/64 sum S'_r sin + S'_i cos
    jj = np.arange(32)
    ph = 2.0 * np.pi * np.outer(f, jj + 32) / M_FFT  # [f, j]
    cp, sp = np.cos(ph) / M_FFT, np.sin(ph) / M_FFT
    wa = np.zeros((128, 64), np.float32)
    wa[0::2, 0::2] = cp
    wa[1::2, 0::2] = -cp
    wa[0::2, 1::2] = sp
    wa[1::2, 1::2] = -sp
    wb = np.zeros((128, 64), np.float32)
    wb[0::2, 0::2] = -sp
    wb[1::2, 0::2] = -sp
    wb[0::2, 1::2] = cp
    wb[1::2, 1::2] = cp
    # lo/hi column variants so every IDFT matmul writes all 128 partitions:
    # even block -> [y; 0], odd block -> [0; y] (accumulated in PSUM).
    wa_lo = np.zeros((128, 128), np.float32)
    wa_lo[:, 0:64] = wa
    wa_hi = np.zeros((128, 128), np.float32)
    wa_hi[:, 64:128] = wa
    wb_lo = np.zeros((128, 128), np.float32)
    wb_lo[:, 0:64] = wb
    wb_hi = np.zeros((128, 128), np.float32)
    wb_hi[:, 64:128] = wb

    ident = np.eye(128, dtype=np.float32)
    return {
        "c_wf": wf.astype(np.float16),
        "c_wh_a": wh_a,
        "c_wh_b": wh_b,
        "c_wa_lo": wa_lo.astype(np.float16),
        "c_wa_hi": wa_hi.astype(np.float16),
        "c_wb_lo": wb_lo.astype(np.float16),
        "c_wb_hi": wb_hi.astype(np.float16),
        "c_ident": ident,
    }


def build_nc(rows=ROWS_PER_CORE, stage=4):
    """Build the per-core Bass program (same NEFF on all 8 cores).

    stage: debug bisection knob. 1 = DMAs + transposes only; 2 = +H path
    (fp32 PE-transpose + fp32 matmuls); 3 = +forward DFT + evac + DVE
    multiplies; 4 = full kernel.
    """
    n_tiles = rows // 128
    nc = bacc.Bacc(
        "TRN2",
        target_bir_lowering=False,
        debug=False,
        enable_asserts=True,
        num_devices=1,
    )
    in_t = nc.dram_tensor("input", [rows, 2 * SMK], F32, kind="ExternalInput").ap()
    cof_t = nc.dram_tensor("cof", [rows, 2 * L], F32, kind="ExternalInput").ap()
    out_t = nc.dram_tensor("output", [rows, 2 * OUT_T], F32, kind="ExternalOutput").ap()
    ht_t = nc.dram_tensor("H_t", [rows, 2 * M_FFT], F32, kind="ExternalOutput").ap()
    cdram = {}
    for name, arr in _constants().items():
        dt = F16 if arr.dtype == np.float16 else F32
        cdram[name] = nc.dram_tensor(name, list(arr.shape), dt, kind="ExternalInput").ap()

    with tile.TileContext(nc) as tc, ExitStack() as ctx:
        cpool = ctx.enter_context(tc.tile_pool(name="consts", bufs=1))
        wf = cpool.tile([128, 128], F16, tag="wf")
        wh_a = cpool.tile([128, 128], F32, tag="wh_a")
        wh_b = cpool.tile([128, 128], F32, tag="wh_b")
        wa_lo = cpool.tile([128, 128], F16, tag="wa_lo")
        wa_hi = cpool.tile([128, 128], F16, tag="wa_hi")
        wb_lo = cpool.tile([128, 128], F16, tag="wb_lo")
        wb_hi = cpool.tile([128, 128], F16, tag="wb_hi")
        ident = cpool.tile([128, 128], F32, tag="ident")
        for tl, name in (
            (wf, "c_wf"),
            (wh_a, "c_wh_a"),
            (wh_b, "c_wh_b"),
            (wa_lo, "c_wa_lo"),
            (wa_hi, "c_wa_hi"),
            (wb_lo, "c_wb_lo"),
            (wb_hi, "c_wb_hi"),
            (ident, "c_ident"),
        ):
            nc.sync.dma_start(tl[:], cdram[name][:])

        rm_pool = ctx.enter_context(tc.tile_pool(name="rm", bufs=2))
        t0_pool = ctx.enter_context(tc.tile_pool(name="t0", bufs=2))
        og_pool = ctx.enter_context(tc.tile_pool(name="og", bufs=2))
        ot_pool = ctx.enter_context(tc.tile_pool(name="ot", bufs=2))
        cof_pool = ctx.enter_context(tc.tile_pool(name="cofp", bufs=2))
        tapsT_pool = ctx.enter_context(tc.tile_pool(name="tapsT", bufs=2))
        h_pool = ctx.enter_context(tc.tile_pool(name="hsb", bufs=2))
        hrep_pool = ctx.enter_context(tc.tile_pool(name="hrep", bufs=2))
        ssb_pool = ctx.enter_context(tc.tile_pool(name="ssb", bufs=3))
        ta_pool = ctx.enter_context(tc.tile_pool(name="ta", bufs=3))
        tb_pool = ctx.enter_context(tc.tile_pool(name="tb", bufs=3))
        s_psum = ctx.enter_context(tc.tile_pool(name="spsum", bufs=2, space="PSUM"))
        o_psum = ctx.enter_context(tc.tile_pool(name="opsum", bufs=2, space="PSUM"))
        h_psum = ctx.enter_context(tc.tile_pool(name="hpsum", bufs=1, space="PSUM"))
        taps_psum = ctx.enter_context(tc.tile_pool(name="tpsum", bufs=1, space="PSUM"))

        for ti in range(n_tiles):
            rsl = slice(ti * 128, ti * 128 + 128)

            # ---- load + pad + transpose signal ----
            rm = rm_pool.tile([128, RME], F16, tag="rm")
            nc.gpsimd.memset(rm[:, 0:64], 0.0)
            nc.gpsimd.memset(rm[:, 4160:RME], 0.0)
            nc.gpsimd.dma_start(rm[:, 64:4160], in_t[rsl, :])  # f32 -> f16 cast
            t0 = t0_pool.tile([128, RME], F16, tag="t0")
            t0_3d = t0[:].rearrange("p (q r) -> p q r", r=128)
            nc.sync.dma_start(t0_3d, rm[:], transpose=True)
            t1 = t0_pool.tile([128, RME - 128], F16, tag="t1")
            t1_3d = t1[:].rearrange("p (q r) -> p q r", r=128)
            nc.sync.dma_start(t1_3d, rm[:, 64 : RME - 64], transpose=True)

            # ---- taps -> H (fp32) ----
            if stage < 2:
                og = og_pool.tile([128, RME], F16, tag="og")
                nc.vector.tensor_copy(og[:], rm[:])
                ot = ot_pool.tile([128, RME], F16, tag="ot")
                ot_3d = ot[:].rearrange("p (q r) -> p q r", r=128)
                nc.sync.dma_start(ot_3d, og[:], transpose=True)
                nc.gpsimd.dma_start(out_t[rsl, :], ot[:, 0 : 2 * OUT_T])
                continue
            cofp = cof_pool.tile([128, 128], F32, tag="cofp")
            nc.gpsimd.memset(cofp[:, 64:128], 0.0)
            nc.sync.dma_start(cofp[:, 0:64], cof_t[rsl, :])
            tp = taps_psum.tile([128, 128], F32, tag="tpsum")
            nc.tensor.transpose(tp[:], cofp[:], ident[:])
            tapsT = tapsT_pool.tile([128, 128], F32, tag="tapsT")
            nc.vector.tensor_copy(tapsT[:], tp[:])
            hp = h_psum.tile([128, 384], F32, tag="hpsum")
            # H_t row-major = tapsT.T @ wh_a ; H1/H2 = wh_{a,b}.T @ tapsT
            nc.tensor.matmul(hp[:, 0:128], tapsT[:], wh_a[:], start=True, stop=False)
            nc.tensor.matmul(hp[:, 128:256], wh_a[:], tapsT[:], start=False, stop=False)
            nc.tensor.matmul(hp[:, 256:384], wh_b[:], tapsT[:], start=False, stop=True)
            ht_sb = h_pool.tile([128, 128], F32, tag="htsb")
            nc.scalar.copy(ht_sb[:], hp[:, 0:128])
            nc.sync.dma_start(ht_t[rsl, :], ht_sb[:])
            h1 = h_pool.tile([128, 128], F16, tag="h1")
            h2 = h_pool.tile([128, 128], F16, tag="h2")
            nc.scalar.copy(h1[:], hp[:, 128:256])
            nc.scalar.copy(h2[:], hp[:, 256:384])
            h1r = hrep_pool.tile([128, SG * 128], F16, tag="h1r")
            h2r = hrep_pool.tile([128, SG * 128], F16, tag="h2r")
            nc.vector.tensor_copy(h1r[:, 0:128], h1[:])
            nc.vector.tensor_copy(h2r[:, 0:128], h2[:])
            w = 128
            while w < SG * 128:
                nc.vector.tensor_copy(h1r[:, w : 2 * w], h1r[:, 0:w])
                nc.vector.tensor_copy(h2r[:, w : 2 * w], h2r[:, 0:w])
                w *= 2

            # ---- output staging (transposed layout) ----
            og = og_pool.tile([128, RME], F16, tag="og")
            if stage < 3:
                nc.vector.tensor_copy(og[:], rm[:])
                ot = ot_pool.tile([128, RME], F16, tag="ot")
                ot_3d = ot[:].rearrange("p (q r) -> p q r", r=128)
                nc.sync.dma_start(ot_3d, og[:], transpose=True)
                nc.gpsimd.dma_start(out_t[rsl, :], ot[:, 0 : 2 * OUT_T])
                continue

            # ---- blocks in supergroups of SG ----
            for g in range((NB + SG - 1) // SG):
                b0 = g * SG
                nblk = min(SG, NB - b0)
                nbank = (nblk + 3) // 4
                sp = s_psum.tile([128, 1024], F32, tag="spsum")
                # forward DFT: one K=128 matmul per block, all with wf.
                # Even blocks read T0 chunk b/2, odd blocks T1 chunk (b-1)/2.
                for bi in range(nblk):
                    b = b0 + bi
                    rhs = t0_3d[:, b // 2, :] if b % 2 == 0 else t1_3d[:, (b - 1) // 2, :]
                    in_bank_i = bi % 4
                    in_bank_n = min(4, nblk - 4 * (bi // 4))
                    nc.tensor.matmul(
                        sp[:, bi * 128 : bi * 128 + 128],
                        wf[:],
                        rhs,
                        start=(in_bank_i == 0),
                        stop=(in_bank_i == in_bank_n - 1),
                    )

                s_sb = ssb_pool.tile([128, 1024], F16, tag="ssb")
                nc.scalar.copy(s_sb[:, 0 : nblk * 128], sp[:, 0 : nblk * 128])
                ta = ta_pool.tile([128, 1024], F16, tag="ta")
                tb = tb_pool.tile([128, 1024], F16, tag="tb")
                nc.vector.tensor_mul(
                    ta[:, 0 : nblk * 128], s_sb[:, 0 : nblk * 128], h1r[:, 0 : nblk * 128]
                )
                nc.vector.tensor_mul(
                    tb[:, 0 : nblk * 128], s_sb[:, 0 : nblk * 128], h2r[:, 0 : nblk * 128]
                )

                if stage < 4:
                    if g < 4:
                        nc.vector.tensor_copy(
                            og[:, g * 1024 : (g + 1) * 1024], ta[:] if g % 2 == 0 else tb[:]
                        )
                    continue

                # inverse DFT (keep-half) with +/- recombination folded in.
                # Every matmul writes all 128 partitions (lo/hi weight
                # variants), so one accumulation group per PSUM bank.
                op = o_psum.tile([128, 512], F32, tag="opsum")
                imms = []
                for wmat, tt, par in (
                    (wa_lo, ta, 0),
                    (wb_lo, tb, 0),
                    (wa_hi, ta, 1),
                    (wb_hi, tb, 1),
                ):
                    for bi in range(nblk):
                        if bi % 2 == par:
                            imms.append((wmat, tt, bi))
                for k, (wmat, tt, bi) in enumerate(imms):
                    csl = slice((bi // 2) * 128, (bi // 2) * 128 + 128)
                    nc.tensor.matmul(
                        op[:, csl],
                        wmat[:],
                        tt[:, bi * 128 : bi * 128 + 128],
                        start=(k == 0),
                        stop=(k == len(imms) - 1),
                    )
                # evacuate pairs into og
                npair = (nblk + 1) // 2
                nc.scalar.copy(
                    og[:, b0 * 64 : b0 * 64 + npair * 128],
                    op[:, 0 : npair * 128],
                )

            # ---- transpose back + store ----
            ot = ot_pool.tile([128, RME], F16, tag="ot")
            ot_3d = ot[:].rearrange("p (q r) -> p q r", r=128)
            nc.sync.dma_start(ot_3d, og[:], transpose=True)
            nc.gpsimd.dma_start(out_t[rsl, :], ot[:, 0 : 2 * OUT_T])  # f16 -> f32

    nc.compile()
    return nc


_NC_CACHE = {}


def _get_nc(rows):
    if rows not in _NC_CACHE:
        _NC_CACHE[rows] = build_nc(rows)
    return _NC_CACHE[rows]


def kernel(input, cof, M=64, **_unused):
    assert int(M) == 64, f"kernel hardcodes M=64, got {M}"
    input = np.ascontiguousarray(np.asarray(input, dtype=np.float32))
    cof = np.ascontiguousarray(np.asarray(cof, dtype=np.float32))
    assert input.shape == (N_FULL, P_DIM, SMK, 2), input.shape
    assert cof.shape == (N_FULL, P_DIM, L, 2), cof.shape

    from concourse.bass_utils import run_bass_kernel_spmd

    nc = _get_nc(ROWS_PER_CORE)
    consts = _constants()
    in_maps = []
    for i in range(N_CORES):
        nsl = slice(i * N_PER, (i + 1) * N_PER)
        m = {
            "input": np.ascontiguousarray(
                input[nsl].reshape(ROWS_PER_CORE, 2 * SMK)
            ),
            "cof": np.ascontiguousarray(cof[nsl].reshape(ROWS_PER_CORE, 2 * L)),
        }
        m.update(consts)
        in_maps.append(m)

    res = run_bass_kernel_spmd(nc, in_maps, list(range(N_CORES)))
    outs = []
    hts = []
    for i in range(N_CORES):
        outs.append(res.results[i]["output"].reshape(N_PER, P_DIM, OUT_T, 2))
        hts.append(res.results[i]["H_t"].reshape(N_PER, P_DIM, M_FFT, 2))
    output = np.concatenate(outs, axis=0)
    h_t = np.concatenate(hts, axis=0)
    return output, h_t


if __name__ == "__main__":
    # quick self-build check
    nc = build_nc(256)
    print("built ok")


# revision 16
# speedup vs baseline: 47547.8473x; 47547.8473x over previous
"""Trainium2 Bass kernel for nn_Channel: per-row complex FIR (overlap-save
DFT-64 on the TensorEngine) + 64-point DFT of the taps (H_t).

Contract: kernel(**inputs) takes the FULL unsharded inputs
  input: (256, 64, 2048, 2) f32, cof: (256, 64, 32, 2) f32, M: 64
returns (output, H_t) with
  output: (256, 64, 2079, 2) f32, H_t: (256, 64, 64, 2) f32
The batch dim N=256 is sharded across 8 NeuronCores (pure data parallel).
Shard/gather on the host also handles the fp32<->fp16 cast and the
time-major <-> partition-major layout swap, so the device streams fp16
in its native compute layout (no on-device transposes).

Algorithm (per row = one (n, p) pair; 2048 rows per core):
  y = conv_full(x, h) via overlap-save with FFT size 64 and hop 32:
    block b covers output times [32b, 32b+32); its input window is
    x[32b-32 .. 32b+32).  DFT-64 / IDFT-64 are batched N=512 matmuls
    with constant real-packed [128x128] fp16 weights on the PE; the
    per-row spectrum product S*H is 2 elementwise tensor_tensor ops on
    the DVE (t_a = S*[Hr;Hi], t_b = S*[Hi;Hr]); the +/- recombination
    is folded into the constant IDFT weights.  Odd blocks straddle
    128-element chunks and use zero-padded full-K weight variants
    (two accumulating matmuls), so only one transposed copy of the
    signal is needed.
  H_t = DFT-64 of zero-padded taps, computed in fp32 (exact to ~1e-7).

Device-side layouts per 128-row tile:
  T0 fp16 [128, 33, 128]: T0[p, q, r] = padded_signal[row r, element 128q+p]
    (elements are (time, re/im) interleaved; 32 zero times padded front/back)
  OG fp16 [128, 33, 128]: OG[e, u, r] = output block-pair u of row r,
    e = 64*(parity) + 2*(t - 32) + comp.
"""

import os
import sys
from contextlib import ExitStack

import numpy as np

for _p in (
    "/root/.axon_site",
    "/root/.axon_site/_ro/trn_rl_repo",
    "/root/.axon_site/_ro/pypackages",
    "/opt/trn_rl_repo",
):
    if os.path.isdir(_p) and _p not in sys.path:
        sys.path.append(_p)

import concourse.bass as bass  # noqa: E402,F401
import concourse.tile as tile  # noqa: E402
from concourse import bacc, mybir  # noqa: E402

F32 = mybir.dt.float32
F16 = mybir.dt.float16

N_CORES = 8
N_FULL, P_DIM, SMK, L, M_FFT = 256, 64, 2048, 32, 64
N_PER = N_FULL // N_CORES  # 32
ROWS_PER_CORE = N_PER * P_DIM  # 2048
N_TILES = ROWS_PER_CORE // 128  # 16
OUT_T = SMK + L - 1  # 2079
NB = 65  # overlap-save blocks per row (hop 32)
NCH = 33  # 128-element chunks in the padded output row
RME = NCH * 128  # 4224 padded row elements
NIC = NB  # input chunks per row (hop-64 overlapping windows, one per block)
IME = NIC * 128  # 8320 input elements per row (2x duplicated)
SG = 8  # blocks per supergroup (2 PSUM banks of S, 1 of O)


def _constants():
    """Constant weight matrices (numpy, fed as ExternalInputs)."""
    f = np.arange(M_FFT)
    t = np.arange(M_FFT)
    th = 2.0 * np.pi * np.outer(t, f) / M_FFT  # [t, f]
    c, s = np.cos(th), np.sin(th)
    # Forward DFT, real-packed.  Row 2t+comp, col 2f+comp'.
    # S_r = sum xr cos + xi sin ; S_i = sum xi cos - xr sin
    wf = np.zeros((128, 128), np.float32)
    wf[0::2, 0::2] = c
    wf[1::2, 0::2] = s
    wf[0::2, 1::2] = -s
    wf[1::2, 1::2] = c
    # Odd blocks read chunk q (local elements 0..63 at partitions 64..127)
    # and chunk q+1 (local elements 64..127 at partitions 0..63); zero-padded
    # full-K variants keep every matmul at base partition 0.
    wodd_a = np.zeros_like(wf)
    wodd_a[64:, :] = wf[:64, :]
    wodd_b = np.zeros_like(wf)
    wodd_b[:64, :] = wf[64:, :]

    # Taps DFT (H): rows are (l, comp), l < 32 (rest zero).
    wh_a = np.zeros((128, 128), np.float32)
    wh_a[0::2, 0::2] = c
    wh_a[1::2, 0::2] = s
    wh_a[0::2, 1::2] = -s
    wh_a[1::2, 1::2] = c
    wh_a[64:, :] = 0.0
    # Swapped variant: col 2f+0 = H_i, col 2f+1 = H_r
    wh_b = np.zeros((128, 128), np.float32)
    wh_b[0::2, 0::2] = -s
    wh_b[1::2, 0::2] = c
    wh_b[0::2, 1::2] = c
    wh_b[1::2, 1::2] = s
    wh_b[64:, :] = 0.0

    # IDFT keep-half weights.  t_a[2f+c'] = (SrHr, SiHi), t_b = (SrHi, SiHr).
    # S'_r = ta[2f]-ta[2f+1], S'_i = tb[2f]+tb[2f+1]
    # y_r[j] = 1/64 sum_f S'_r cos - S'_i sin ; y_i = 1/64 sum S'_r sin + S'_i cos
    jj = np.arange(32)
    ph = 2.0 * np.pi * np.outer(f, jj + 32) / M_FFT  # [f, j]
    cp, sp = np.cos(ph) / M_FFT, np.sin(ph) / M_FFT
    wa = np.zeros((128, 64), np.float32)
    wa[0::2, 0::2] = cp
    wa[1::2, 0::2] = -cp
    wa[0::2, 1::2] = sp
    wa[1::2, 1::2] = -sp
    wb = np.zeros((128, 64), np.float32)
    wb[0::2, 0::2] = -sp
    wb[1::2, 0::2] = -sp
    wb[0::2, 1::2] = cp
    wb[1::2, 1::2] = cp
    # lo/hi column variants so every IDFT matmul writes all 128 partitions:
    # even block -> [y; 0], odd block -> [0; y] (accumulated in PSUM).
    wa_lo = np.zeros((128, 128), np.float32)
    wa_lo[:, 0:64] = wa
    wa_hi = np.zeros((128, 128), np.float32)
    wa_hi[:, 64:128] = wa
    wb_lo = np.zeros((128, 128), np.float32)
    wb_lo[:, 0:64] = wb
    wb_hi = np.zeros((128, 128), np.float32)
    wb_hi[:, 64:128] = wb

    ident = np.eye(128, dtype=np.float32)
    return {
        "c_wf": wf.astype(np.float16),
        "c_wodd_a": wodd_a.astype(np.float16),
        "c_wodd_b": wodd_b.astype(np.float16),
        "c_wh_a": wh_a,
        "c_wh_b": wh_b,
        "c_wa_lo": wa_lo.astype(np.float16),
        "c_wa_hi": wa_hi.astype(np.float16),
        "c_wb_lo": wb_lo.astype(np.float16),
        "c_wb_hi": wb_hi.astype(np.float16),
        "c_ident": ident,
    }


def build_nc(n_tiles=N_TILES):
    """Build the per-core Bass program (same NEFF on all 8 cores)."""
    rows = n_tiles * 128
    nc = bacc.Bacc(
        "TRN2",
        target_bir_lowering=False,
        debug=False,
        enable_asserts=True,
        num_devices=1,
    )
    # input in hop-64 window layout fp16: [tile, p, b*128+r]
    in_t = nc.dram_tensor("x_t0", [n_tiles, 128, IME], F16, kind="ExternalInput").ap()
    cof_t = nc.dram_tensor("cof", [rows, 2 * L], F32, kind="ExternalInput").ap()
    # output in OG layout fp16: [tile, e, u*128+r]
    out_t = nc.dram_tensor("y_og", [n_tiles, 128, RME], F16, kind="ExternalOutput").ap()
    ht_t = nc.dram_tensor("H_t", [rows, 2 * M_FFT], F32, kind="ExternalOutput").ap()
    cdram = {}
    carrs = _constants()
    for name, arr in carrs.items():
        dt = F16 if arr.dtype == np.float16 else F32
        cdram[name] = nc.dram_tensor(name, list(arr.shape), dt, kind="ExternalInput").ap()

    with tile.TileContext(nc) as tc, ExitStack() as ctx:
        cpool = ctx.enter_context(tc.tile_pool(name="consts", bufs=1))
        tiles_c = {}
        for name, arr in carrs.items():
            dt = F16 if arr.dtype == np.float16 else F32
            tl = cpool.tile(list(arr.shape), dt, tag=name)
            nc.sync.dma_start(tl[:], cdram[name][:])
            tiles_c[name] = tl
        wf = tiles_c["c_wf"]
        wodd_a = tiles_c["c_wodd_a"]
        wodd_b = tiles_c["c_wodd_b"]
        wh_a = tiles_c["c_wh_a"]
        wh_b = tiles_c["c_wh_b"]
        wa_lo = tiles_c["c_wa_lo"]
        wa_hi = tiles_c["c_wa_hi"]
        wb_lo = tiles_c["c_wb_lo"]
        wb_hi = tiles_c["c_wb_hi"]
        ident = tiles_c["c_ident"]

        t0_pool = ctx.enter_context(tc.tile_pool(name="t0", bufs=3))
        og_pool = ctx.enter_context(tc.tile_pool(name="og", bufs=3))
        cof_pool = ctx.enter_context(tc.tile_pool(name="cofp", bufs=2))
        tapsT_pool = ctx.enter_context(tc.tile_pool(name="tapsT", bufs=2))
        h_pool = ctx.enter_context(tc.tile_pool(name="hsb", bufs=2))
        hrep_pool = ctx.enter_context(tc.tile_pool(name="hrep", bufs=2))
        ssb_pool = ctx.enter_context(tc.tile_pool(name="ssb", bufs=4))
        ta_pool = ctx.enter_context(tc.tile_pool(name="ta", bufs=4))
        tb_pool = ctx.enter_context(tc.tile_pool(name="tb", bufs=4))
        s_psum = ctx.enter_context(tc.tile_pool(name="spsum", bufs=2, space="PSUM"))
        o_psum = ctx.enter_context(tc.tile_pool(name="opsum", bufs=2, space="PSUM"))
        h_psum = ctx.enter_context(tc.tile_pool(name="hpsum", bufs=1, space="PSUM"))
        taps_psum = ctx.enter_context(tc.tile_pool(name="tpsum", bufs=1, space="PSUM"))

        for ti in range(n_tiles):
            rsl = slice(ti * 128, ti * 128 + 128)

            # ---- load transposed signal (one 128-el window per block) ----
            t0 = t0_pool.tile([128, IME], F16, tag="t0")
            t0_3d = t0[:].rearrange("p (q r) -> p q r", r=128)
            nc.sync.dma_start(t0[:], in_t[ti, :, :])

            # ---- taps -> H (fp32) ----
            cofp = cof_pool.tile([128, 128], F32, tag="cofp")
            nc.gpsimd.memset(cofp[:, 64:128], 0.0)
            nc.gpsimd.dma_start(cofp[:, 0:64], cof_t[rsl, :])
            tp = taps_psum.tile([128, 128], F32, tag="tpsum")
            nc.tensor.transpose(tp[:], cofp[:], ident[:])
            tapsT = tapsT_pool.tile([128, 128], F32, tag="tapsT")
            nc.vector.tensor_copy(tapsT[:], tp[:])
            hp = h_psum.tile([128, 384], F32, tag="hpsum")
            # H_t row-major = tapsT.T @ wh_a ; H1/H2 = wh_{a,b}.T @ tapsT
            nc.tensor.matmul(hp[:, 0:128], tapsT[:], wh_a[:], start=True, stop=False)
            nc.tensor.matmul(hp[:, 128:256], wh_a[:], tapsT[:], start=False, stop=False)
            nc.tensor.matmul(hp[:, 256:384], wh_b[:], tapsT[:], start=False, stop=True)
            ht_sb = h_pool.tile([128, 128], F32, tag="htsb")
            nc.scalar.copy(ht_sb[:], hp[:, 0:128])
            nc.gpsimd.dma_start(ht_t[rsl, :], ht_sb[:])
            h1 = h_pool.tile([128, 128], F16, tag="h1")
            h2 = h_pool.tile([128, 128], F16, tag="h2")
            nc.scalar.copy(h1[:], hp[:, 128:256])
            nc.scalar.copy(h2[:], hp[:, 256:384])
            h1r = hrep_pool.tile([128, 1024], F16, tag="h1r")
            h2r = hrep_pool.tile([128, 1024], F16, tag="h2r")
            nc.vector.tensor_copy(h1r[:, 0:128], h1[:])
            nc.vector.tensor_copy(h2r[:, 0:128], h2[:])
            w = 128
            while w < 1024:
                nc.vector.tensor_copy(h1r[:, w : 2 * w], h1r[:, 0:w])
                nc.vector.tensor_copy(h2r[:, w : 2 * w], h2r[:, 0:w])
                w *= 2

            # ---- output staging ----
            og = og_pool.tile([128, RME], F16, tag="og")
            # lone trailing block leaves the odd half of slot 32 unwritten
            nc.gpsimd.memset(og[64:128, 4096:RME], 0.0)

            # ---- blocks in supergroups of SG ----
            # Natural block order: block b0+bi at S columns bi*128.  Forward
            # DFT = 2 batched N=512 matmuls (all with wf).  IDFT uses
            # even/odd strided views of ta/tb with lo/hi weight variants.
            for g in range((NB + SG - 1) // SG):
                b0 = g * SG
                nblk = min(SG, NB - b0)
                ne = (nblk + 1) // 2
                no = nblk // 2
                span = nblk * 128
                sp = s_psum.tile([128, 1024], F32, tag="spsum")
                n1 = min(nblk, 4)
                nc.tensor.matmul(
                    sp[:, 0 : n1 * 128], wf[:], t0_3d[:, b0 : b0 + n1, :],
                    start=True, stop=True,
                )
                if nblk > 4:
                    nc.tensor.matmul(
                        sp[:, 512 : span], wf[:], t0_3d[:, b0 + 4 : b0 + nblk, :],
                        start=True, stop=True,
                    )

                s_sb = ssb_pool.tile([128, 1024], F16, tag="ssb")
                nc.scalar.copy(s_sb[:, 0:span], sp[:, 0:span])
                ta = ta_pool.tile([128, 1024], F16, tag="ta")
                tb = tb_pool.tile([128, 1024], F16, tag="tb")
                nc.vector.tensor_mul(ta[:, 0:span], s_sb[:, 0:span], h1r[:, 0:span])
                nc.vector.tensor_mul(tb[:, 0:span], s_sb[:, 0:span], h2r[:, 0:span])
                ta3 = ta[:].rearrange("p (u two r) -> p two u r", two=2, r=128)
                tb3 = tb[:].rearrange("p (u two r) -> p two u r", two=2, r=128)

                # inverse DFT (keep-half): batched matmuls accumulate into one
                # O bank; even blocks write [y;0], odd blocks [0;y].
                op = o_psum.tile([128, 512], F32, tag="opsum")
                nc.tensor.matmul(
                    op[:, 0 : ne * 128], wa_lo[:], ta3[:, 0, 0:ne, :],
                    start=True, stop=False,
                )
                nc.tensor.matmul(
                    op[:, 0 : ne * 128], wb_lo[:], tb3[:, 0, 0:ne, :],
                    start=False, stop=(no == 0),
                )
                if no:
                    nc.tensor.matmul(
                        op[:, 0 : no * 128], wa_hi[:], ta3[:, 1, 0:no, :],
                        start=False, stop=False,
                    )
                    nc.tensor.matmul(
                        op[:, 0 : no * 128], wb_hi[:], tb3[:, 1, 0:no, :],
                        start=False, stop=True,
                    )
                if g % 2 == 0:
                    nc.scalar.copy(
                        og[:, b0 * 64 : b0 * 64 + ne * 128], op[:, 0 : ne * 128]
                    )
                else:
                    nc.vector.tensor_copy(
                        og[:, b0 * 64 : b0 * 64 + ne * 128], op[:, 0 : ne * 128]
                    )

            # ---- store ----
            nc.sync.dma_start(out_t[ti, :, :], og[:])

    nc.compile()
    return nc


_NC_CACHE = {}


def _get_nc(n_tiles=N_TILES):
    if n_tiles not in _NC_CACHE:
        _NC_CACHE[n_tiles] = build_nc(n_tiles)
    return _NC_CACHE[n_tiles]


def host_pack_input(input_f32):
    """(Nslice, P, SMK, 2) f32 -> hop-64 window layout fp16
    [n_tiles, 128, NIC*128]: window b holds padded elements [64b, 64b+128)."""
    rows = input_f32.shape[0] * P_DIM
    nt = rows // 128
    pad = np.zeros((rows, RME), np.float16)
    pad[:, 64:4160] = input_f32.reshape(rows, 2 * SMK).astype(np.float16)
    # windows[r, b, l] = pad[r, 64b + l]
    sw = np.lib.stride_tricks.sliding_window_view(pad, 128, axis=1)[:, ::64, :]
    assert sw.shape == (rows, NIC, 128), sw.shape
    # T[t, p, b, r] = windows[t*128+r, b, p]
    t0 = np.ascontiguousarray(sw.reshape(nt, 128, NIC, 128).transpose(0, 3, 2, 1))
    return t0.reshape(nt, 128, IME)


def host_unpack_output(y_og):
    """OG-layout fp16 [n_tiles, 128, RME] -> (rows, 2*OUT_T) f32."""
    nt = y_og.shape[0]
    # og[t, e, u, r] -> rows[t*128+r, 128u+e]
    o = y_og.reshape(nt, 128, NCH, 128).transpose(0, 3, 2, 1)
    o = np.ascontiguousarray(o).reshape(nt * 128, RME)
    return o[:, 0 : 2 * OUT_T].astype(np.float32)


def kernel(input, cof, M=64, **_unused):
    assert int(M) == 64, f"kernel hardcodes M=64, got {M}"
    input = np.asarray(input, dtype=np.float32)
    cof = np.asarray(cof, dtype=np.float32)
    assert input.shape == (N_FULL, P_DIM, SMK, 2), input.shape
    assert cof.shape == (N_FULL, P_DIM, L, 2), cof.shape

    from concourse.bass_utils import run_bass_kernel_spmd

    nc = _get_nc(N_TILES)
    consts = _constants()
    in_maps = []
    for i in range(N_CORES):
        nsl = slice(i * N_PER, (i + 1) * N_PER)
        m = {
            "x_t0": host_pack_input(input[nsl]),
            "cof": np.ascontiguousarray(cof[nsl].reshape(ROWS_PER_CORE, 2 * L)),
        }
        m.update(consts)
        in_maps.append(m)

    res = run_bass_kernel_spmd(nc, in_maps, list(range(N_CORES)))
    outs = []
    hts = []
    for i in range(N_CORES):
        y = host_unpack_output(res.results[i]["y_og"])
        outs.append(y.reshape(N_PER, P_DIM, OUT_T, 2))
        hts.append(res.results[i]["H_t"].reshape(N_PER, P_DIM, M_FFT, 2))
    output = np.concatenate(outs, axis=0)
    h_t = np.concatenate(hts, axis=0)
    return output, h_t


if __name__ == "__main__":
    nc = build_nc(2)
    print("built ok")
